# revision 10
# baseline (speedup 1.0000x reference)
"""Trainium2 Bass kernel for an nn.Block dense transformer layer.

Reference computation (per batch element b of 8):
    x = x + MHA(LN1(x));  x = x + MLP(LN2(x))
with T=1024 tokens, C=512 channels, H=16 heads (d=32), MLP hidden 2048,
new-gelu (tanh approx), softmax without causal mask.

Sharding: pure data parallelism - each of the 8 NeuronCores processes one
batch element. No collectives.

v2 redesign (from the 260us/body baseline): the kernel is elementwise-
bound, not matmul-bound - ScalarE(ACT) was ~170us busy (softmax exp is
16.8M elements at 1 elem/lane/cycle) and DVE ~105us. Changes:
  - Softmax exp is split across TWO engines per key-tile: ACT Exp
    (fp8 out) and a single DVE tensor_scalar that computes the fp8e4m3
    BIT PATTERN directly (Schraudolph in the log2 domain:
    bits = round(8*(log2e*scale*s + log2(SA) + 7 + sigma)), fp32->uint8
    convert saturates [0,255], uint8 tile bitcast to fp8). Host-side
    simulation shows identical rel-err (1.175e-2) at any mix.
  - LN1/LN2 weight+bias folded host-side into wqkv/wfc (+biases), so
    the LN tail is (x-mu)*rstd16 -> fp8 with NO ACT affine op; rstd16
    = exp(-0.5*ln(var+eps) + ln16) keeps Ln/Exp on the shared table.
  - LN square and subtract run on GPSIMD (idle otherwise); stats use
    fused scalar_tensor_tensor ops.
  - q/k descale folded into the exp scale (q,k carry SX*SW each);
    evac is an ACT Identity with host-prescaled bias.
  - AV dn/rcp/normalize tail batches 2 heads per [64,2,512] PSUM tile:
    one reciprocal_approx_fast + one normalize TT per pair (halves DVE
    tail time); placement DMAs ride the idle SP HWDGE queue.
  - MLP of chunk 0 is emitted inside chunk 1's attention (PE has slack
    under the exp stream); MLP of chunk 1 drains at the end.

Scale scheme (validated vs reference in numpy, rel ~1.18e-2 < 2e-2):
  LN outs x16 (SX via rstd bias ln16); weights x4096 (cproj x8192);
  q,k carry F=SX*SW=2^16 each, exp scale = SCALE/F^2; exp out = 8*exp;
  v8 = 16*v; dn ones = 0.5 so av/dn = 32*y; descales in evac ops.
"""

import sys

if "/opt/trn_rl_repo" not in sys.path:
    sys.path.insert(0, "/opt/trn_rl_repo")

import math
from contextlib import ExitStack

import ml_dtypes
import numpy as np

import concourse.bass as bass
import concourse.mybir as mybir
import concourse.tile as tile
from concourse import bacc
from concourse import bass_utils

F32 = mybir.dt.float32
F32R = mybir.dt.float32r
BF16 = mybir.dt.bfloat16
F8 = mybir.dt.float8e4
U8 = mybir.dt.uint8
AF = mybir.ActivationFunctionType
OP = mybir.AluOpType
DR = mybir.MatmulPerfMode.DoubleRow

N_CORES = 8
T = 1024  # tokens
C = 512  # channels
H = 16  # heads
D = 32  # head dim
FF = 2048  # mlp hidden
CT = C // 128  # channel partition tiles (4)
FT = FF // 128  # mlp hidden partition tiles (16)
NQ = T // 512  # token (query) 512-chunks (2)
G = H // 4  # head groups of 4 (4)
EPS = 1e-5
SCALE = 1.0 / math.sqrt(D)

SX = 16.0  # LN-output fp8 scale (applied via rstd16)
SW = 4096.0  # qkv/proj/fc weight scale
SWC = 8192.0  # cproj weight scale
SA = 8.0  # exp-output scale
SV = 16.0  # v fp8 scale
ONES_VAL = 0.5  # dn ones value -> av/dn = (SA*SV)/(SA*ONES_VAL) * y = 32*y
SY = SV / ONES_VAL  # 32
F_QK = SX * SW  # factor carried by q and k each (2^16)
D_V = SV / (SX * SW)  # 2^-12
D_PROJ = 1.0 / (SY * SW)  # 2^-17
D_FC = 1.0 / (SX * SW)  # 2^-16
D_CPROJ = 1.0 / (1.0 * SWC)  # 2^-13 (gelu out stored unscaled)
GELU_FUNC = AF.Gelu_apprx_tanh
LN16 = math.log(16.0)
LN_SA = math.log(SA)
EXP_SCALE = SCALE / (F_QK * F_QK)
# Schraudolph fp8e4m3-bits exp on DVE: uint8 = sat(round(A8*s + B8)),
# bit pattern == fp8(8*exp(s*SCALE_eff)) within ~4% (softmax cancels it).
SIGMA = 0.043
SCH_A8 = 8.0 * math.log2(math.e) * EXP_SCALE
SCH_B8 = 8.0 * (math.log2(SA) + 7.0 + SIGMA)

# ---- tuning knobs ----
# (ktg, i) tile indices (0..15 = 2*ktg+i) routed to the DVE exp lane.
EXP_DVE_IDX = frozenset({2, 5, 8, 11, 14})
QK_EVAC_ENG = "scalar"  # "scalar" (ACT) or "vector"
LN_SQ_ENG = "gpsimd"
LN_SUB_ENG = "gpsimd"
RCP_ON_ACT = False  # move pair-reciprocal to ACT as exp(-ln(dn))


def r32(ap):
    return ap.bitcast(F32R)


class _NS:
    pass


def emit_prep(ctx, nc, tc, io, tag=""):
    """Persistent tiles + weight/const DMAs. x is loaded FIRST."""
    P = _NS()
    wpool = ctx.enter_context(tc.tile_pool(name="w" + tag, bufs=1))

    def single(shape, dtype, t):
        return wpool.tile(shape, dtype, tag=t, name=t)

    # ---- activations (persistent) ----
    x_t = [single([128, T], F32, f"xT{k}") for k in range(CT)]
    a8 = single([128, CT, T], F8, "a8")  # LN out *16, DR-paired layout
    q_t = [single([128, T], BF16, f"qT{g}") for g in range(G)]
    k_t = [single([128, T], BF16, f"kT{g}") for g in range(G)]
    # v8: [token, kt, head, 64] with cols 0:32 = 0.5 (dn-ones), cols
    # 32:64 = v*16: one M=64 DR matmul yields dn rows 0:32 + av rows
    # 32:64 in the pair PSUM tile.
    v8 = single([128, 8, H, 64], F8, "v8")
    av8 = single([128, G, T], F8, "av8")  # y*32, DR-paired for proj

    # x first so LN1 can start while weights stream in
    for nt in range(NQ):
        for k in range(CT):
            nc.sync.dma_start(
                out=x_t[k].bitcast(F32R)[:, 512 * nt : 512 * (nt + 1)],
                in_=io["xT"].bitcast(F32R)[
                    128 * k : 128 * (k + 1), 512 * nt : 512 * (nt + 1)
                ],
            )

    # ---- fp8 weights (DR-paired layout [p, kt, out_features]) ----
    w_qkv = single([128, CT, 3 * C], F8, "wqkv8")
    w_proj = single([128, CT, C], F8, "wproj8")
    w_fc = single([128, CT, FF], F8, "wfc8")
    w_cproj = single([128, FT, C], F8, "wcproj8")
    nc.sync.dma_start(out=w_qkv, in_=io["wqkv8"])
    nc.scalar.dma_start(out=w_fc, in_=io["wfc8"])
    nc.scalar.dma_start(out=w_cproj, in_=io["wcproj8"])
    nc.scalar.dma_start(out=w_proj, in_=io["wproj8"])

    # ---- bias columns: tile[p, m] = vec[m*128 + p] ----
    def colmat(dram_ap, ntiles, t):
        tl = single([128, ntiles], F32, t)
        nc.sync.dma_start(out=tl, in_=dram_ap.transpose([1, 0]))
        return tl

    b_qk = colmat(io["bqk"], 8, "bqk")  # pre-scaled *F_QK host-side
    b_proj = colmat(io["bproj"], CT, "bproj")
    b_fc = colmat(io["bfc"], FT, "bfc")
    b_cproj = colmat(io["bcproj"], CT, "bcproj")

    # v bias broadcast (*16) to all partitions [128, C]
    bv_bc = single([128, C], F32, "bv_bc")
    nc.gpsimd.dma_start(
        out=bv_bc,
        in_=bass.AP(tensor=io["bv16"].tensor, offset=0, ap=[[0, 128], [1, C]]),
    )

    ones_f = single([128, 128], F32, "ones_f")
    nc.sync.dma_start(out=ones_f.bitcast(F32R), in_=io["ones_d"].bitcast(F32R))
    nc.vector.memset(v8, ONES_VAL)  # evac overwrites the v halves
    eps_t = single([128, 1], F32, "eps_t")
    nc.vector.memset(eps_t, EPS)
    ln16_t = single([128, 1], F32, "ln16_t")
    nc.vector.memset(ln16_t, LN16)
    lnsa_t = single([128, 1], F32, "lnsa_t")
    nc.vector.memset(lnsa_t, LN_SA)

    # rotating pools (SBUF)
    tmp = ctx.enter_context(tc.tile_pool(name="tmp" + tag, bufs=3))
    stat = ctx.enter_context(tc.tile_pool(name="stat" + tag, bufs=2))
    a2p = ctx.enter_context(tc.tile_pool(name="a2p" + tag, bufs=2))
    g8p = ctx.enter_context(tc.tile_pool(name="g8p" + tag, bufs=1))
    rcpp = ctx.enter_context(tc.tile_pool(name="rcp" + tag, bufs=2))
    y8p = ctx.enter_context(tc.tile_pool(name="y8p" + tag, bufs=2))
    dnp = ctx.enter_context(tc.tile_pool(name="dnp" + tag, bufs=2))

    for name in ("x_t", "a8", "q_t", "k_t", "v8", "av8", "w_qkv", "w_proj",
                 "w_fc", "w_cproj", "b_qk", "b_proj", "b_fc", "b_cproj",
                 "bv_bc", "ones_f", "eps_t", "ln16_t", "lnsa_t",
                 "tmp", "stat", "a2p", "g8p", "rcpp", "y8p", "dnp"):
        setattr(P, name, locals()[name])
    return P


def emit_body(nc, tc, io, P, tag="", reload_x=False):
    p = P
    eng = lambda name: getattr(nc, name)
    if reload_x:
        for k in range(CT):
            nc.sync.dma_start(
                out=p.x_t[k].bitcast(F32R),
                in_=io["xT"].bitcast(F32R)[128 * k : 128 * (k + 1), :],
            )

    # ---------------- LayerNorm (transposed domain) -> a8 fp8 -------------
    def layernorm(cols, psp):
        """LN over channel (partition) axis of x_t restricted to token
        range `cols`; writes (x-mu)*rstd16 as fp8 into a8[:, k, cols].
        LN w/b are folded into the consumer weights host-side."""
        ncols = cols.stop - cols.start
        musum = psp.tile([128, ncols], F32, tag="mm", name="ln_mu")
        sqsum = psp.tile([128, ncols], F32, tag="mm", name="ln_sq")
        for k in range(CT):
            sq = p.tmp.tile([128, ncols], F32, tag="sq", name="sq")
            eng(LN_SQ_ENG).tensor_tensor(
                out=sq.bitcast(F32R), in0=p.x_t[k][:, cols],
                in1=p.x_t[k][:, cols], op=OP.mult,
            )
            nc.tensor.matmul(
                out=musum, lhsT=r32(p.ones_f), rhs=r32(p.x_t[k][:, cols]),
                start=(k == 0), stop=(k == CT - 1),
            )
            nc.tensor.matmul(
                out=sqsum, lhsT=r32(p.ones_f), rhs=r32(sq),
                start=(k == 0), stop=(k == CT - 1),
            )
        mu = p.stat.tile([128, ncols], F32, tag="mu", name="mu")
        rstd = p.stat.tile([128, ncols], F32, tag="rstd", name="rstd")
        var = p.stat.tile([128, ncols], F32, tag="var", name="var")
        nc.vector.tensor_scalar_mul(out=mu, in0=musum, scalar1=1.0 / C)
        # mu^2 in SBUF (PSUM allows only one DVE read port), then
        # var = sqsum/C - mu^2 with a single PSUM input.
        nc.vector.tensor_tensor(out=rstd, in0=mu, in1=mu, op=OP.mult)
        nc.vector.scalar_tensor_tensor(
            out=var, in0=sqsum, scalar=1.0 / C, in1=rstd,
            op0=OP.mult, op1=OP.subtract,
        )
        # rstd16 = exp(-0.5*ln(var+eps) + ln16) (natural_log_exp table set)
        nc.scalar.activation(out=var, in_=var, func=AF.Ln, bias=p.eps_t,
                             scale=1.0)
        nc.scalar.activation(out=rstd, in_=var, func=AF.Exp, bias=p.ln16_t,
                             scale=-0.5)
        for k in range(CT):
            t1 = p.tmp.tile([128, ncols], F32, tag="t1", name="ln_t1")
            eng(LN_SUB_ENG).tensor_tensor(
                out=t1.bitcast(F32R), in0=p.x_t[k][:, cols], in1=mu,
                op=OP.subtract,
            )
            nc.vector.tensor_tensor(
                out=p.a8[:, k, cols], in0=t1, in1=rstd, op=OP.mult
            )

    # ======================= LN1 + QKV (DR fp8) ===========================
    with tc.tile_pool(name="ps1" + tag, bufs=4, space="PSUM") as pmm:
        # q^T, k^T: transposed out (feature on partitions), bf16 + bias.
        for nt in range(NQ):
            layernorm(slice(512 * nt, 512 * (nt + 1)), pmm)
            # nt0: q/k pairs per head group; nt1: k tiles first so head
            # group g's scores (which need k over all T) unblock earliest.
            m_order = (0, 4, 1, 5, 2, 6, 3, 7) if nt == 0 else (4, 0, 5, 1, 6, 2, 7, 3)
            for m in m_order:
                dst = p.q_t[m] if m < 4 else p.k_t[m - 4]
                ps = pmm.tile([128, 512], F32, tag="mm", name="qk_ps")
                for j in range(2):
                    nc.tensor.matmul(
                        out=ps,
                        lhsT=p.w_qkv[:, 2 * j : 2 * j + 2, 128 * m : 128 * (m + 1)],
                        rhs=p.a8[:, 2 * j : 2 * j + 2, 512 * nt : 512 * (nt + 1)],
                        start=(j == 0), stop=(j == 1), perf_mode=DR,
                    )
                if QK_EVAC_ENG == "scalar":
                    nc.scalar.activation(
                        out=dst[:, 512 * nt : 512 * (nt + 1)], in_=ps,
                        func=AF.Identity, bias=p.b_qk[:, m : m + 1], scale=1.0,
                    )
                else:
                    bqk_bc = bass.AP(
                        tensor=p.b_qk.tensor, offset=p.b_qk[:, m : m + 1].offset,
                        ap=[list(p.b_qk.ap[0]), [0, 512]],
                    )
                    nc.vector.scalar_tensor_tensor(
                        out=dst[:, 512 * nt : 512 * (nt + 1)], in0=ps,
                        scalar=1.0, in1=bqk_bc, op0=OP.mult, op1=OP.add,
                    )
            # v natural layout [token, vfeat]: lhsT = a8 token-tile
            for t in range(4 * nt, 4 * nt + 4):
                ps = pmm.tile([128, C], F32, tag="mm", name="v_ps")
                for j in range(2):
                    nc.tensor.matmul(
                        out=ps,
                        lhsT=p.a8[:, 2 * j : 2 * j + 2, 128 * t : 128 * (t + 1)],
                        rhs=p.w_qkv[:, 2 * j : 2 * j + 2, 2 * C : 3 * C],
                        start=(j == 0), stop=(j == 1), perf_mode=DR,
                    )
                nc.vector.scalar_tensor_tensor(
                    out=p.v8[:, t, :, 32:64], in0=ps, scalar=D_V, in1=p.bv_bc,
                    op0=OP.mult, op1=OP.add,
                )

    # =========================== Attention ================================
    # per (qc, g): scores (bf16, 4-head row-packed) -> exp via ACT or DVE
    # (fp8 bits) -> A2; per head pair one [64,2,512] PSUM tile collects
    # dn rows 0:32 + av rows 32:64 for both heads; one rcp + one TT.
    with tc.tile_pool(name="sc" + tag, bufs=1, space="PSUM") as scp, \
         tc.tile_pool(name="avdn" + tag, bufs=1, space="PSUM") as avp, \
         tc.tile_pool(name="mid" + tag, bufs=2, space="PSUM") as midp:
        sc_ctr = [0]

        def proj_ln2(qc):
            """proj + residual + LN2 for chunk qc; emitted mid-attention."""
            qs = slice(512 * qc, 512 * (qc + 1))
            for m in range(CT):
                ps = midp.tile([128, 512], F32, tag="mm", name="proj_ps")
                for j in range(2):
                    nc.tensor.matmul(
                        out=ps,
                        lhsT=p.w_proj[:, 2 * j : 2 * j + 2, 128 * m : 128 * (m + 1)],
                        rhs=p.av8[:, 2 * j : 2 * j + 2, qs],
                        start=(j == 0), stop=(j == 1), perf_mode=DR,
                    )
                nc.vector.affine_then_add(
                    out=p.x_t[m][:, qs].bitcast(F32R), in0=ps,
                    in1=p.x_t[m][:, qs], scale=D_PROJ,
                    bias=p.b_proj[:, m : m + 1],
                )
            layernorm(qs, midp)

        g8_tiles = {}

        def emit_mlp(qc, pmm, m_range=None, do_cproj=True):
            """FC+gelu for fc tiles in m_range; cproj+residual+store if
            do_cproj. g8 chunk tile is allocated once per chunk (tile()
            with a repeated tag would rotate to a fresh buffer)."""
            qs = slice(512 * qc, 512 * (qc + 1))
            if qc not in g8_tiles:
                g8_tiles[qc] = p.g8p.tile([128, FT, 512], F8,
                                          tag=f"g8_{qc}", name="g8")
            g8 = g8_tiles[qc]
            for m in (m_range if m_range is not None else range(FT)):
                ps = pmm.tile([128, 512], F32, tag="mm", name="fc_ps")
                for j in range(2):
                    nc.tensor.matmul(
                        out=ps,
                        lhsT=p.w_fc[:, 2 * j : 2 * j + 2, 128 * m : 128 * (m + 1)],
                        rhs=p.a8[:, 2 * j : 2 * j + 2, qs],
                        start=(j == 0), stop=(j == 1), perf_mode=DR,
                    )
                nc.scalar.activation(
                    out=g8[:, m, :], in_=ps, func=GELU_FUNC,
                    bias=p.b_fc[:, m : m + 1], scale=D_FC,
                )
            if not do_cproj:
                return
            for m in range(CT):
                ps = pmm.tile([128, 512], F32, tag="mm", name="cproj_ps")
                for j in range(FT // 2):
                    nc.tensor.matmul(
                        out=ps,
                        lhsT=p.w_cproj[:, 2 * j : 2 * j + 2, 128 * m : 128 * (m + 1)],
                        rhs=g8[:, 2 * j : 2 * j + 2, :],
                        start=(j == 0), stop=(j == FT // 2 - 1), perf_mode=DR,
                    )
                nc.vector.affine_then_add(
                    out=p.x_t[m][:, qs].bitcast(F32R), in0=ps,
                    in1=p.x_t[m][:, qs], scale=D_CPROJ,
                    bias=p.b_cproj[:, m : m + 1],
                )
                nc.sync.dma_start(
                    out=io["yT"][128 * m : 128 * (m + 1), qs],
                    in_=p.x_t[m][:, qs],
                )

        # Software pipeline: group g's A.V work is interleaved into group
        # g+1's scores stream so the exp lanes never wait on AV bursts.
        av_state = {}

        def emit_av_step(a2_prev, g_prev, qs_prev, step):
            h, j = divmod(step, 4)
            pair, slot = divmod(h, 2)
            hg = 4 * g_prev + h
            if slot == 0 and j == 0:
                av_state["t"] = avp.tile([64, 2, 512], F32, tag="av",
                                         name="av_ps")
            avt = av_state["t"]
            nc.tensor.matmul(
                out=avt[:, slot, :],
                lhsT=p.v8[:, 2 * j : 2 * j + 2, hg, :],
                rhs=a2_prev[h // 2].bitcast(F8)[
                    :, 2 * j : 2 * j + 2, 512 * (h % 2) : 512 * (h % 2) + 512,
                ],
                start=(j == 0), stop=(j == 3), perf_mode=DR,
            )
            if slot == 1 and j == 3:
                # dn rows 0:32 (both heads) -> one rcp; shift to av rows by
                # DMA; one normalize TT; per-head placement DMA into av8.
                rcpt = p.rcpp.tile([32, 2, 512], F32, tag="rcp", name="rcp")
                if RCP_ON_ACT:
                    lnt = p.rcpp.tile([32, 2, 512], F32, tag="lnt", name="lnt")
                    nc.scalar.activation(out=lnt, in_=avt[0:32, :, :],
                                         func=AF.Ln, bias=0.0, scale=1.0)
                    nc.scalar.activation(out=rcpt, in_=lnt, func=AF.Exp,
                                         bias=0.0, scale=-1.0)
                else:
                    nc.vector.reciprocal_approx_fast(
                        out=rcpt, in_=avt[0:32, :, :])
                rcps = p.dnp.tile([64, 2, 512], F32, tag="rcps", name="rcps")
                nc.sync.dma_start(out=rcps[32:64, :, :], in_=rcpt)
                y8s = p.y8p.tile([64, 2, 512], F8, tag="y8", name="y8s")
                nc.vector.tensor_tensor(
                    out=y8s[32:64, :, :], in0=avt[32:64, :, :],
                    in1=rcps[32:64, :, :], op=OP.mult,
                )
                for sl in range(2):
                    hh = 2 * pair + sl
                    nc.sync.dma_start(
                        out=p.av8[32 * hh : 32 * hh + 32, g_prev, qs_prev],
                        in_=y8s[32:64, sl, :],
                    )

        prev = None
        for qc in range(NQ):
            qs = slice(512 * qc, 512 * (qc + 1))
            for g in range(G):
                a2 = [p.a2p.tile([128, 8, 1024], U8, tag=f"a2_{i}", name="a2")
                      for i in range(2)]
                for half in range(2):
                    for kt in range(4):
                        ktg = 4 * half + kt
                        sc = []
                        for i in range(2):
                            t2 = sc_ctr[0] % 2
                            sc_ctr[0] += 1
                            sc.append(scp.tile([128, 1024], F32,
                                               tag=f"sc{t2}", name="sc"))
                        for c in range(4):
                            pr = slice(32 * c, 32 * (c + 1))
                            nc.tensor.matmul(
                                out=sc[c // 2][:, 512 * (c % 2) : 512 * (c % 2 + 1)],
                                lhsT=p.k_t[g][pr, 128 * ktg : 128 * (ktg + 1)],
                                rhs=p.q_t[g][pr, qs],
                                start=True, stop=True,
                                tile_position=(32 * c, 0),
                            )
                        for i in range(2):
                            if (2 * ktg + i) in EXP_DVE_IDX:
                                nc.vector.tensor_scalar(
                                    out=a2[i][:, ktg, :], in0=sc[i],
                                    scalar1=SCH_A8, scalar2=SCH_B8,
                                    op0=OP.mult, op1=OP.add,
                                )
                            else:
                                nc.scalar.activation(
                                    out=a2[i].bitcast(F8)[:, ktg, :],
                                    in_=sc[i], func=AF.Exp,
                                    bias=p.lnsa_t, scale=EXP_SCALE,
                                )
                if prev is not None:
                    for _ in range(16):
                        emit_av_step(*prev)
                        prev = (prev[0], prev[1], prev[2], prev[3] + 1)
                    if qc == 1 and g == 1:
                        # chunk 0's AV fully drained during (qc1, g0)
                        proj_ln2(0)
                        emit_mlp(0, midp, m_range=range(0, 6), do_cproj=False)
                    elif qc == 1 and g == 2:
                        emit_mlp(0, midp, m_range=range(6, 12), do_cproj=False)
                    elif qc == 1 and g == 3:
                        emit_mlp(0, midp, m_range=range(12, 16), do_cproj=False)
                prev = (a2, g, qs, 0)
        # drain the last group's AV work
        for _ in range(16):
            emit_av_step(*prev)
            prev = (prev[0], prev[1], prev[2], prev[3] + 1)
        emit_mlp(0, midp, m_range=[], do_cproj=True)
        proj_ln2(1)

    # ======================== MLP chunk 1 (DR fp8) ========================
    with tc.tile_pool(name="ps2" + tag, bufs=4, space="PSUM") as pmm:
        emit_mlp(1, pmm)


def emit_block(ctx, nc, tc, io, tag="", repeats=1):
    P = emit_prep(ctx, nc, tc, io, tag)
    for r in range(repeats):
        emit_body(nc, tc, io, P, tag + f"r{r}" if r else tag, reload_x=(r > 0))


def declare_io(nc):
    def inp(name, shape, dtype=F32):
        return nc.dram_tensor(name, shape, dtype, kind="ExternalInput").ap()

    io = {
        "xT": inp("xT", [C, T]),
        "wqkv8": inp("wqkv8", [128, CT, 3 * C], F8),
        "wproj8": inp("wproj8", [128, CT, C], F8),
        "wfc8": inp("wfc8", [128, CT, FF], F8),
        "wcproj8": inp("wcproj8", [128, FT, C], F8),
        "bqk": inp("bqk", [8, 128]),
        "bv16": inp("bv16", [1, C]),
        "bproj": inp("bproj", [CT, 128]),
        "bfc": inp("bfc", [FT, 128]),
        "bcproj": inp("bcproj", [CT, 128]),
        "ones_d": inp("ones_d", [128, 128]),
        "yT": nc.dram_tensor("yT", [C, T], F32, kind="ExternalOutput").ap(),
    }
    return io


def build(num_devices=N_CORES, repeats=1):
    nc = bacc.Bacc(
        "TRN2", target_bir_lowering=False, debug=False, num_devices=num_devices
    )
    # Pin Exp to the natural_log_exp table set (shared with Ln): the
    # default per-function set choice thrashes ACT_TABLE_LOADs between
    # exp_and_others and natural_log_exp on every LayerNorm.
    import concourse.hw_specs as _hws

    _tabs = _hws.get_activation_tables(nc.m.arch)
    for _name in ("exp_and_others", "exp_and_friends"):
        if _name in _tabs:
            _tabs[_name].clear()
    io = declare_io(nc)
    with tile.TileContext(nc) as tc, ExitStack() as ctx:
        emit_block(ctx, nc, tc, io, repeats=repeats)
    nc.compile()
    return nc


def _w8(w_t, scale):
    """[K, M] transposed weight -> DR-paired fp8 [128, K//128, M]."""
    f8 = mybir.dt.np(F8)
    k, m = w_t.shape
    return np.ascontiguousarray(
        (w_t * scale).reshape(k // 128, 128, m).transpose(1, 0, 2)
    ).astype(f8)


def host_inputs(x_b, attn_w, attn_b, proj_w, proj_b, fc_w, fc_b, cproj_w, cproj_b,
                ln1_w, ln1_b, ln2_w, ln2_b):
    """Per-core input dict for batch element x_b [T, C]. Folds LN1 w/b
    into wqkv/biases and LN2 w/b into wfc/biases."""
    f = np.float32
    wqkv = attn_w * ln1_w[None, :]
    bqkv = attn_b + attn_w @ ln1_b
    wfc = fc_w * ln2_w[None, :]
    bfc = fc_b + fc_w @ ln2_b
    return {
        "xT": np.ascontiguousarray(x_b.T, dtype=f),
        "wqkv8": _w8(wqkv.T.astype(f), SW),
        "wproj8": _w8(proj_w.T.astype(f), SW),
        "wfc8": _w8(wfc.T.astype(f), SW),
        "wcproj8": _w8(cproj_w.T.astype(f), SWC),
        "bqk": np.ascontiguousarray(
            (bqkv[: 2 * C] * F_QK).reshape(8, 128), dtype=f),
        "bv16": np.ascontiguousarray(
            (bqkv[2 * C :] * SV).reshape(1, C), dtype=f),
        "bproj": np.ascontiguousarray(proj_b.reshape(CT, 128), dtype=f),
        "bfc": np.ascontiguousarray(bfc.reshape(FT, 128), dtype=f),
        "bcproj": np.ascontiguousarray(cproj_b.reshape(CT, 128), dtype=f),
        "ones_d": np.ones((128, 128), dtype=f),
    }


def unpack_output(result_map):
    """Map one core's output tensors to the [T, C] batch element."""
    return result_map["yT"].T


_CACHED_NC = None
_LAST_RES = None


def kernel(x, ln1_w, ln1_b, attn_w, attn_b, proj_w, proj_b,
           ln2_w, ln2_b, fc_w, fc_b, cproj_w, cproj_b):
    global _CACHED_NC, _LAST_RES
    x = np.asarray(x)
    B = x.shape[0]
    assert B == N_CORES and x.shape[1] == T and x.shape[2] == C
    if _CACHED_NC is None:
        _CACHED_NC = build()
    nc = _CACHED_NC
    args = [np.asarray(a, dtype=np.float32)
            for a in (attn_w, attn_b, proj_w, proj_b, fc_w, fc_b,
                      cproj_w, cproj_b, ln1_w, ln1_b, ln2_w, ln2_b)]
    (attn_w, attn_b, proj_w, proj_b, fc_w, fc_b,
     cproj_w, cproj_b, ln1_w, ln1_b, ln2_w, ln2_b) = args
    in_maps = [
        host_inputs(x[b], attn_w, attn_b, proj_w, proj_b, fc_w, fc_b,
                    cproj_w, cproj_b, ln1_w, ln1_b, ln2_w, ln2_b)
        for b in range(B)
    ]
    res = bass_utils.run_bass_kernel_spmd(
        nc, in_maps, core_ids=list(range(N_CORES))
    )
    _LAST_RES = res
    out = np.empty((B, T, C), np.float32)
    for b in range(B):
        out[b] = unpack_output(res.results[b])
    return out


# revision 33
# speedup vs baseline: 1.0397x; 1.0397x over previous
"""Trainium2 Bass kernel for an nn.Block dense transformer layer.

Reference computation (per batch element b of 8):
    x = x + MHA(LN1(x));  x = x + MLP(LN2(x))
with T=1024 tokens, C=512 channels, H=16 heads (d=32), MLP hidden 2048,
new-gelu (tanh approx), softmax without causal mask.

Sharding: pure data parallelism - each of the 8 NeuronCores processes one
batch element. No collectives.

v2 redesign (from the 260us/body baseline; measured ~200us with the
end-tile DVE exp lane, ~227-230us without): the kernel
is elementwise-bound, not matmul-bound - ScalarE(ACT) is the pacing
engine (softmax exp is 16.8M elements at 1 elem/lane/cycle ~ 128us
minimum) with DVE second. Changes vs baseline:
  - LN1/LN2 weight+bias folded host-side into wqkv/wfc (+biases), so
    the LN tail is (x-mu)*rstd16 -> fp8 with NO ACT affine op; rstd16
    = exp(-0.5*ln(var+eps) + ln16) keeps Ln/Exp on the shared table.
  - LN square and subtract run on GPSIMD (idle otherwise); stats use
    a fused scalar_tensor_tensor.
  - q/k descale folded into the exp scale (q,k carry SX*SW each);
    evac is an ACT Identity with host-prescaled bias (no descale mult).
  - AV dn/rcp/normalize tail batches 2 heads per [64,2,512] PSUM tile:
    one reciprocal_approx_fast + one normalize TT per pair (halves DVE
    tail time); shift/placement DMAs ride the SP HWDGE queue.
  - MLP of chunk 0 is emitted inside chunk 1's attention (PE has slack
    under the exp stream); MLP of chunk 1 drains at the end.
  - A second softmax-exp lane on the DVE: one tensor_scalar computes
    the fp8e4m3 BIT PATTERN directly (Schraudolph in the log2 domain,
    bits = round(8*(log2e*scale*s + log2(SA)+7+sigma)), saturating
    fp32->uint8 convert, uint8 tile bitcast to fp8; rel err identical
    to ACT exp, op microbenches ~1.1us/[128,1024] tile). Routing is
    position-critical: ONLY the end-of-group tiles (ktg 6,7) are
    routed to the DVE - by then the DVE's AV-tail for the previous
    group has drained, so the lane keeps pace with the 2-deep
    score-tag rotation. Mid-stream routing serializes against the
    PE's in-order FIFO (+~8us per routed tile at every granularity
    tried: per-tile, per-unit a2 tiles, drain reordering, per-head
    tails). Measured ~200us vs ~227-230us all-ACT in same-session
    conditions.

Scale scheme (validated vs reference in numpy, rel ~1.18e-2 < 2e-2):
  LN outs x16 (SX via rstd bias ln16); weights x4096 (cproj x8192);
  q,k carry F=SX*SW=2^16 each, exp scale = SCALE/F^2; exp out = 8*exp;
  v8 = 16*v; dn ones = 0.5 so av/dn = 32*y; descales in evac ops.
"""

import sys

if "/opt/trn_rl_repo" not in sys.path:
    sys.path.insert(0, "/opt/trn_rl_repo")

import math
from contextlib import ExitStack

import ml_dtypes
import numpy as np

import concourse.bass as bass
import concourse.mybir as mybir
import concourse.tile as tile
from concourse import bacc
from concourse import bass_utils

F32 = mybir.dt.float32
F32R = mybir.dt.float32r
BF16 = mybir.dt.bfloat16
F8 = mybir.dt.float8e4
U8 = mybir.dt.uint8
AF = mybir.ActivationFunctionType
OP = mybir.AluOpType
DR = mybir.MatmulPerfMode.DoubleRow

N_CORES = 8
T = 1024  # tokens
C = 512  # channels
H = 16  # heads
D = 32  # head dim
FF = 2048  # mlp hidden
CT = C // 128  # channel partition tiles (4)
FT = FF // 128  # mlp hidden partition tiles (16)
NQ = T // 512  # token (query) 512-chunks (2)
G = H // 4  # head groups of 4 (4)
EPS = 1e-5
SCALE = 1.0 / math.sqrt(D)

SX = 16.0  # LN-output fp8 scale (applied via rstd16)
SW = 4096.0  # qkv/proj/fc weight scale
SWC = 8192.0  # cproj weight scale
SA = 8.0  # exp-output scale
SV = 16.0  # v fp8 scale
ONES_VAL = 0.5  # dn ones value -> av/dn = (SA*SV)/(SA*ONES_VAL) * y = 32*y
SY = SV / ONES_VAL  # 32
F_QK = SX * SW  # factor carried by q and k each (2^16)
D_V = SV / (SX * SW)  # 2^-12
D_PROJ = 1.0 / (SY * SW)  # 2^-17
D_FC = 1.0 / (SX * SW)  # 2^-16
D_CPROJ = 1.0 / (1.0 * SWC)  # 2^-13 (gelu out stored unscaled)
GELU_FUNC = AF.Gelu_apprx_tanh
LN16 = math.log(16.0)
LN_SA = math.log(SA)
EXP_SCALE = SCALE / (F_QK * F_QK)
# Schraudolph fp8e4m3-bits exp on DVE: uint8 = sat(round(A8*s + B8)),
# bit pattern == fp8(8*exp(s*SCALE_eff)) within ~4% (softmax cancels it).
SIGMA = 0.043
SCH_A8 = 8.0 * math.log2(math.e) * EXP_SCALE
SCH_B8 = 8.0 * (math.log2(SA) + 7.0 + SIGMA)
# fp32-bits Schraudolph (DVE int32 out + gpsimd fp32->fp8 cast pass)
SCH_A32 = float(2.0**23) * math.log2(math.e) * EXP_SCALE
SCH_B32 = float(2.0**23) * (127.0 + math.log2(SA) - 0.0579)

# ---- tuning knobs ----
# (ktg, i) tile indices (0..15 = 2*ktg+i) routed to the DVE exp lane.
def _idxset(name, default):
    return frozenset(
        int(v) for v in __import__("os").environ.get(name, default).split(",")
        if v not in ("", "none")
    )


# End-of-group score tiles (ktg 6,7 both head-pairs) go to the DVE
# fp8-bits lane: by then the DVE's AV-tail work for the previous group
# has drained, so the second exp lane runs without stalling the PE's
# score-tag rotation (mid-stream routing measured ~8us/tile slower).
EXP_DVE_IDX = _idxset("EXP_DVE", "12,13,14,15")
EXP_GPS_IDX = _idxset("EXP_GPS", "none")  # DVE int32 bits + gps fp8 cast
_env = __import__("os").environ.get
QK_EVAC_ENG = _env("QK_EVAC", "scalar")  # "scalar" (ACT) or "vector"
LN_SQ_ENG = _env("LN_SQ", "gpsimd")
LN_SUB_ENG = _env("LN_SUB", "gpsimd")
RCP_ON_ACT = _env("RCP_ACT", "0") == "1"
MLP0_IN_ATTN = _env("MLP0_ATTN", "1") == "1"


def r32(ap):
    return ap.bitcast(F32R)


class _NS:
    pass


def emit_prep(ctx, nc, tc, io, tag=""):
    """Persistent tiles + weight/const DMAs. x is loaded FIRST."""
    P = _NS()
    wpool = ctx.enter_context(tc.tile_pool(name="w" + tag, bufs=1))

    def single(shape, dtype, t):
        return wpool.tile(shape, dtype, tag=t, name=t)

    # ---- activations (persistent) ----
    x_t = [single([128, T], F32, f"xT{k}") for k in range(CT)]
    a8 = single([128, CT, T], F8, "a8")  # LN out *16, DR-paired layout
    q_t = [single([128, T], BF16, f"qT{g}") for g in range(G)]
    k_t = [single([128, T], BF16, f"kT{g}") for g in range(G)]
    # v8: [token, kt, head, 64] with cols 0:32 = 0.5 (dn-ones), cols
    # 32:64 = v*16: one M=64 DR matmul yields dn rows 0:32 + av rows
    # 32:64 in the pair PSUM tile.
    v8 = single([128, 8, H, 64], F8, "v8")
    av8 = single([128, G, T], F8, "av8")  # y*32, DR-paired for proj

    # x first so LN1 can start while weights stream in
    for nt in range(NQ):
        for k in range(CT):
            nc.sync.dma_start(
                out=x_t[k].bitcast(F32R)[:, 512 * nt : 512 * (nt + 1)],
                in_=io["xT"].bitcast(F32R)[
                    128 * k : 128 * (k + 1), 512 * nt : 512 * (nt + 1)
                ],
            )

    # ---- fp8 weights (DR-paired layout [p, kt, out_features]) ----
    w_qkv = single([128, CT, 3 * C], F8, "wqkv8")
    w_proj = single([128, CT, C], F8, "wproj8")
    w_fc = single([128, CT, FF], F8, "wfc8")
    w_cproj = single([128, FT, C], F8, "wcproj8")
    nc.sync.dma_start(out=w_qkv, in_=io["wqkv8"])
    nc.scalar.dma_start(out=w_fc, in_=io["wfc8"])
    nc.scalar.dma_start(out=w_cproj, in_=io["wcproj8"])
    nc.scalar.dma_start(out=w_proj, in_=io["wproj8"])

    # ---- bias columns: tile[p, m] = vec[m*128 + p] ----
    def colmat(dram_ap, ntiles, t):
        tl = single([128, ntiles], F32, t)
        nc.sync.dma_start(out=tl, in_=dram_ap.transpose([1, 0]))
        return tl

    b_qk = colmat(io["bqk"], 8, "bqk")  # pre-scaled *F_QK host-side
    b_proj = colmat(io["bproj"], CT, "bproj")
    b_fc = colmat(io["bfc"], FT, "bfc")
    b_cproj = colmat(io["bcproj"], CT, "bcproj")

    # v bias broadcast (*16) to all partitions [128, C]
    bv_bc = single([128, C], F32, "bv_bc")
    nc.gpsimd.dma_start(
        out=bv_bc,
        in_=bass.AP(tensor=io["bv16"].tensor, offset=0, ap=[[0, 128], [1, C]]),
    )

    ones_f = single([128, 128], F32, "ones_f")
    nc.sync.dma_start(out=ones_f.bitcast(F32R), in_=io["ones_d"].bitcast(F32R))
    nc.vector.memset(v8, ONES_VAL)  # evac overwrites the v halves
    eps_t = single([128, 1], F32, "eps_t")
    nc.vector.memset(eps_t, EPS)
    ln16_t = single([128, 1], F32, "ln16_t")
    nc.vector.memset(ln16_t, LN16)
    lnsa_t = single([128, 1], F32, "lnsa_t")
    nc.vector.memset(lnsa_t, LN_SA)

    # rotating pools (SBUF)
    tmp = ctx.enter_context(tc.tile_pool(name="tmp" + tag, bufs=3))
    stat = ctx.enter_context(tc.tile_pool(name="stat" + tag, bufs=2))
    a2p = ctx.enter_context(tc.tile_pool(name="a2p" + tag, bufs=2))
    g8p = ctx.enter_context(tc.tile_pool(name="g8p" + tag, bufs=1))
    rcpp = ctx.enter_context(tc.tile_pool(name="rcp" + tag, bufs=2))
    y8p = ctx.enter_context(tc.tile_pool(name="y8p" + tag, bufs=2))
    dnp = ctx.enter_context(tc.tile_pool(name="dnp" + tag, bufs=2))
    schp = ctx.enter_context(tc.tile_pool(name="schp" + tag, bufs=2))

    for name in ("x_t", "a8", "q_t", "k_t", "v8", "av8", "w_qkv", "w_proj",
                 "w_fc", "w_cproj", "b_qk", "b_proj", "b_fc", "b_cproj",
                 "bv_bc", "ones_f", "eps_t", "ln16_t", "lnsa_t",
                 "tmp", "stat", "a2p", "g8p", "rcpp", "y8p", "dnp", "schp"):
        setattr(P, name, locals()[name])
    return P


def emit_body(nc, tc, io, P, tag="", reload_x=False):
    p = P
    eng = lambda name: getattr(nc, name)
    if reload_x:
        for k in range(CT):
            nc.sync.dma_start(
                out=p.x_t[k].bitcast(F32R),
                in_=io["xT"].bitcast(F32R)[128 * k : 128 * (k + 1), :],
            )

    # ---------------- LayerNorm (transposed domain) -> a8 fp8 -------------
    def layernorm(cols, psp):
        """LN over channel (partition) axis of x_t restricted to token
        range `cols`; writes (x-mu)*rstd16 as fp8 into a8[:, k, cols].
        LN w/b are folded into the consumer weights host-side."""
        ncols = cols.stop - cols.start
        musum = psp.tile([128, ncols], F32, tag="mm", name="ln_mu")
        sqsum = psp.tile([128, ncols], F32, tag="mm", name="ln_sq")
        for k in range(CT):
            sq = p.tmp.tile([128, ncols], F32, tag="sq", name="sq")
            eng(LN_SQ_ENG).tensor_tensor(
                out=sq.bitcast(F32R), in0=p.x_t[k][:, cols],
                in1=p.x_t[k][:, cols], op=OP.mult,
            )
            nc.tensor.matmul(
                out=musum, lhsT=r32(p.ones_f), rhs=r32(p.x_t[k][:, cols]),
                start=(k == 0), stop=(k == CT - 1),
            )
            nc.tensor.matmul(
                out=sqsum, lhsT=r32(p.ones_f), rhs=r32(sq),
                start=(k == 0), stop=(k == CT - 1),
            )
        mu = p.stat.tile([128, ncols], F32, tag="mu", name="mu")
        rstd = p.stat.tile([128, ncols], F32, tag="rstd", name="rstd")
        var = p.stat.tile([128, ncols], F32, tag="var", name="var")
        nc.vector.tensor_scalar_mul(out=mu, in0=musum, scalar1=1.0 / C)
        # mu^2 in SBUF (PSUM allows only one DVE read port), then
        # var = sqsum/C - mu^2 with a single PSUM input.
        nc.vector.tensor_tensor(out=rstd, in0=mu, in1=mu, op=OP.mult)
        nc.vector.scalar_tensor_tensor(
            out=var, in0=sqsum, scalar=1.0 / C, in1=rstd,
            op0=OP.mult, op1=OP.subtract,
        )
        # rstd16 = exp(-0.5*ln(var+eps) + ln16) (natural_log_exp table set)
        nc.scalar.activation(out=var, in_=var, func=AF.Ln, bias=p.eps_t,
                             scale=1.0)
        nc.scalar.activation(out=rstd, in_=var, func=AF.Exp, bias=p.ln16_t,
                             scale=-0.5)
        for k in range(CT):
            t1 = p.tmp.tile([128, ncols], F32, tag="t1", name="ln_t1")
            eng(LN_SUB_ENG).tensor_tensor(
                out=t1.bitcast(F32R), in0=p.x_t[k][:, cols], in1=mu,
                op=OP.subtract,
            )
            nc.vector.tensor_tensor(
                out=p.a8[:, k, cols], in0=t1, in1=rstd, op=OP.mult
            )

    # ======================= LN1 + QKV (DR fp8) ===========================
    with tc.tile_pool(name="ps1" + tag, bufs=4, space="PSUM") as pmm:
        # q^T, k^T: transposed out (feature on partitions), bf16 + bias.
        for nt in range(NQ):
            layernorm(slice(512 * nt, 512 * (nt + 1)), pmm)
            # nt0: q/k pairs per head group; nt1: k tiles first so head
            # group g's scores (which need k over all T) unblock earliest.
            m_order = (0, 4, 1, 5, 2, 6, 3, 7) if nt == 0 else (4, 0, 5, 1, 6, 2, 7, 3)
            for m in m_order:
                dst = p.q_t[m] if m < 4 else p.k_t[m - 4]
                ps = pmm.tile([128, 512], F32, tag="mm", name="qk_ps")
                for j in range(2):
                    nc.tensor.matmul(
                        out=ps,
                        lhsT=p.w_qkv[:, 2 * j : 2 * j + 2, 128 * m : 128 * (m + 1)],
                        rhs=p.a8[:, 2 * j : 2 * j + 2, 512 * nt : 512 * (nt + 1)],
                        start=(j == 0), stop=(j == 1), perf_mode=DR,
                    )
                if QK_EVAC_ENG == "scalar":
                    nc.scalar.activation(
                        out=dst[:, 512 * nt : 512 * (nt + 1)], in_=ps,
                        func=AF.Identity, bias=p.b_qk[:, m : m + 1], scale=1.0,
                    )
                else:
                    bqk_bc = bass.AP(
                        tensor=p.b_qk.tensor, offset=p.b_qk[:, m : m + 1].offset,
                        ap=[list(p.b_qk.ap[0]), [0, 512]],
                    )
                    nc.vector.scalar_tensor_tensor(
                        out=dst[:, 512 * nt : 512 * (nt + 1)], in0=ps,
                        scalar=1.0, in1=bqk_bc, op0=OP.mult, op1=OP.add,
                    )
            # v natural layout [token, vfeat]: lhsT = a8 token-tile
            for t in range(4 * nt, 4 * nt + 4):
                ps = pmm.tile([128, C], F32, tag="mm", name="v_ps")
                for j in range(2):
                    nc.tensor.matmul(
                        out=ps,
                        lhsT=p.a8[:, 2 * j : 2 * j + 2, 128 * t : 128 * (t + 1)],
                        rhs=p.w_qkv[:, 2 * j : 2 * j + 2, 2 * C : 3 * C],
                        start=(j == 0), stop=(j == 1), perf_mode=DR,
                    )
                nc.vector.scalar_tensor_tensor(
                    out=p.v8[:, t, :, 32:64], in0=ps, scalar=D_V, in1=p.bv_bc,
                    op0=OP.mult, op1=OP.add,
                )

    # =========================== Attention ================================
    # per (qc, g): scores (bf16, 4-head row-packed) -> exp via ACT or DVE
    # (fp8 bits) -> A2; per head pair one [64,2,512] PSUM tile collects
    # dn rows 0:32 + av rows 32:64 for both heads; one rcp + one TT.
    with tc.tile_pool(name="sc" + tag, bufs=1, space="PSUM") as scp, \
         tc.tile_pool(name="avdn" + tag, bufs=1, space="PSUM") as avp, \
         tc.tile_pool(name="mid" + tag, bufs=2, space="PSUM") as midp:
        sc_ctr = [0]

        def proj_ln2(qc):
            """proj + residual + LN2 for chunk qc; emitted mid-attention."""
            qs = slice(512 * qc, 512 * (qc + 1))
            for m in range(CT):
                ps = midp.tile([128, 512], F32, tag="mm", name="proj_ps")
                for j in range(2):
                    nc.tensor.matmul(
                        out=ps,
                        lhsT=p.w_proj[:, 2 * j : 2 * j + 2, 128 * m : 128 * (m + 1)],
                        rhs=p.av8[:, 2 * j : 2 * j + 2, qs],
                        start=(j == 0), stop=(j == 1), perf_mode=DR,
                    )
                nc.vector.affine_then_add(
                    out=p.x_t[m][:, qs].bitcast(F32R), in0=ps,
                    in1=p.x_t[m][:, qs], scale=D_PROJ,
                    bias=p.b_proj[:, m : m + 1],
                )
            layernorm(qs, midp)

        g8_tiles = {}

        def emit_mlp(qc, pmm, m_range=None, do_cproj=True):
            """FC+gelu for fc tiles in m_range; cproj+residual+store if
            do_cproj. g8 chunk tile is allocated once per chunk (tile()
            with a repeated tag would rotate to a fresh buffer)."""
            qs = slice(512 * qc, 512 * (qc + 1))
            if qc not in g8_tiles:
                g8_tiles[qc] = p.g8p.tile([128, FT, 512], F8,
                                          tag=f"g8_{qc}", name="g8")
            g8 = g8_tiles[qc]
            for m in (m_range if m_range is not None else range(FT)):
                ps = pmm.tile([128, 512], F32, tag="mm", name="fc_ps")
                for j in range(2):
                    nc.tensor.matmul(
                        out=ps,
                        lhsT=p.w_fc[:, 2 * j : 2 * j + 2, 128 * m : 128 * (m + 1)],
                        rhs=p.a8[:, 2 * j : 2 * j + 2, qs],
                        start=(j == 0), stop=(j == 1), perf_mode=DR,
                    )
                nc.scalar.activation(
                    out=g8[:, m, :], in_=ps, func=GELU_FUNC,
                    bias=p.b_fc[:, m : m + 1], scale=D_FC,
                )
            if not do_cproj:
                return
            for m in range(CT):
                ps = pmm.tile([128, 512], F32, tag="mm", name="cproj_ps")
                for j in range(FT // 2):
                    nc.tensor.matmul(
                        out=ps,
                        lhsT=p.w_cproj[:, 2 * j : 2 * j + 2, 128 * m : 128 * (m + 1)],
                        rhs=g8[:, 2 * j : 2 * j + 2, :],
                        start=(j == 0), stop=(j == FT // 2 - 1), perf_mode=DR,
                    )
                nc.vector.affine_then_add(
                    out=p.x_t[m][:, qs].bitcast(F32R), in0=ps,
                    in1=p.x_t[m][:, qs], scale=D_CPROJ,
                    bias=p.b_cproj[:, m : m + 1],
                )
                nc.sync.dma_start(
                    out=io["yT"][128 * m : 128 * (m + 1), qs],
                    in_=p.x_t[m][:, qs],
                )

        # Software pipeline: group g's A.V work is interleaved into group
        # g+1's scores stream so the exp lanes never wait on AV bursts.
        av_state = {}

        def emit_av_step(a2_prev, g_prev, qs_prev, step):
            h, j = divmod(step, 4)
            pair, slot = divmod(h, 2)
            hg = 4 * g_prev + h
            if slot == 0 and j == 0:
                av_state[pair] = avp.tile([64, 2, 512], F32, tag="av",
                                          name="av_ps")
            avt = av_state[pair]
            off = 512 * (h % 2)
            nc.tensor.matmul(
                out=avt[:, slot, :],
                lhsT=p.v8[:, 2 * j : 2 * j + 2, hg, :],
                rhs=a2_prev[h // 2].bitcast(F8)[
                    :, 2 * j : 2 * j + 2, off : off + 512
                ],
                start=(j == 0), stop=(j == 3), perf_mode=DR,
            )
            if slot == 1 and j == 3:
                # dn rows 0:32 (both heads) -> one rcp; shift to av rows
                # by DMA; one normalize TT; per-head placement DMA.
                rcpt = p.rcpp.tile([32, 2, 512], F32, tag="rcp", name="rcp")
                if RCP_ON_ACT:
                    lnt = p.rcpp.tile([32, 2, 512], F32, tag="lnt", name="lnt")
                    nc.scalar.activation(out=lnt, in_=avt[0:32, :, :],
                                         func=AF.Ln, bias=0.0, scale=1.0)
                    nc.scalar.activation(out=rcpt, in_=lnt, func=AF.Exp,
                                         bias=0.0, scale=-1.0)
                else:
                    nc.vector.reciprocal_approx_fast(
                        out=rcpt, in_=avt[0:32, :, :])
                rcps = p.dnp.tile([64, 2, 512], F32, tag="rcps", name="rcps")
                nc.sync.dma_start(out=rcps[32:64, :, :], in_=rcpt)
                y8s = p.y8p.tile([64, 2, 512], F8, tag="y8", name="y8s")
                nc.vector.tensor_tensor(
                    out=y8s[32:64, :, :], in0=avt[32:64, :, :],
                    in1=rcps[32:64, :, :], op=OP.mult,
                )
                for sl in range(2):
                    hh = 2 * pair + sl
                    nc.sync.dma_start(
                        out=p.av8[32 * hh : 32 * hh + 32, g_prev, qs_prev],
                        in_=y8s[32:64, sl, :],
                    )

        prev = None
        for qc in range(NQ):
            qs = slice(512 * qc, 512 * (qc + 1))
            for g in range(G):
                a2 = [p.a2p.tile([128, 8, 1024], U8, tag=f"a2_{i}", name="a2")
                      for i in range(2)]
                for half in range(2):
                    for kt in range(4):
                        ktg = 4 * half + kt
                        sc = []
                        for i in range(2):
                            t2 = sc_ctr[0] % 2
                            sc_ctr[0] += 1
                            sc.append(scp.tile([128, 1024], F32,
                                               tag=f"sc{t2}", name="sc"))
                        for c in range(4):
                            pr = slice(32 * c, 32 * (c + 1))
                            nc.tensor.matmul(
                                out=sc[c // 2][:, 512 * (c % 2) : 512 * (c % 2 + 1)],
                                lhsT=p.k_t[g][pr, 128 * ktg : 128 * (ktg + 1)],
                                rhs=p.q_t[g][pr, qs],
                                start=True, stop=True,
                                tile_position=(32 * c, 0),
                            )
                        for i in range(2):
                            tile_id = 2 * ktg + i
                            if tile_id in EXP_GPS_IDX:
                                ti = p.schp.tile(
                                    [128, 1024], mybir.dt.int32,
                                    tag="sch", name="sch",
                                )
                                nc.vector.tensor_scalar(
                                    out=ti, in0=sc[i], scalar1=SCH_A32,
                                    scalar2=SCH_B32, op0=OP.mult, op1=OP.add,
                                )
                                nc.gpsimd.tensor_copy(
                                    a2[i].bitcast(F8)[:, ktg, :],
                                    ti.bitcast(F32),
                                )
                            elif tile_id in EXP_DVE_IDX:
                                nc.vector.tensor_scalar(
                                    out=a2[i][:, ktg, :], in0=sc[i],
                                    scalar1=SCH_A8, scalar2=SCH_B8,
                                    op0=OP.mult, op1=OP.add,
                                )
                            else:
                                nc.scalar.activation(
                                    out=a2[i].bitcast(F8)[:, ktg, :],
                                    in_=sc[i], func=AF.Exp,
                                    bias=p.lnsa_t, scale=EXP_SCALE,
                                )
                if prev is not None:
                    for _ in range(16):
                        emit_av_step(*prev)
                        prev = (prev[0], prev[1], prev[2], prev[3] + 1)
                    if qc == 1 and g == 1:
                        # chunk 0's AV fully drained during (qc1, g0)
                        proj_ln2(0)
                        if MLP0_IN_ATTN:
                            emit_mlp(0, midp, m_range=range(0, 6),
                                     do_cproj=False)
                    elif qc == 1 and g == 2 and MLP0_IN_ATTN:
                        emit_mlp(0, midp, m_range=range(6, 12), do_cproj=False)
                    elif qc == 1 and g == 3 and MLP0_IN_ATTN:
                        emit_mlp(0, midp, m_range=range(12, 16), do_cproj=False)
                prev = (a2, g, qs, 0)
        # drain the last group's AV work
        for _ in range(16):
            emit_av_step(*prev)
            prev = (prev[0], prev[1], prev[2], prev[3] + 1)
        if MLP0_IN_ATTN:
            emit_mlp(0, midp, m_range=[], do_cproj=True)
        else:
            emit_mlp(0, midp)
        proj_ln2(1)

    # ======================== MLP chunk 1 (DR fp8) ========================
    with tc.tile_pool(name="ps2" + tag, bufs=4, space="PSUM") as pmm:
        emit_mlp(1, pmm)


def emit_block(ctx, nc, tc, io, tag="", repeats=1):
    P = emit_prep(ctx, nc, tc, io, tag)
    for r in range(repeats):
        emit_body(nc, tc, io, P, tag + f"r{r}" if r else tag, reload_x=(r > 0))


def declare_io(nc):
    def inp(name, shape, dtype=F32):
        return nc.dram_tensor(name, shape, dtype, kind="ExternalInput").ap()

    io = {
        "xT": inp("xT", [C, T]),
        "wqkv8": inp("wqkv8", [128, CT, 3 * C], F8),
        "wproj8": inp("wproj8", [128, CT, C], F8),
        "wfc8": inp("wfc8", [128, CT, FF], F8),
        "wcproj8": inp("wcproj8", [128, FT, C], F8),
        "bqk": inp("bqk", [8, 128]),
        "bv16": inp("bv16", [1, C]),
        "bproj": inp("bproj", [CT, 128]),
        "bfc": inp("bfc", [FT, 128]),
        "bcproj": inp("bcproj", [CT, 128]),
        "ones_d": inp("ones_d", [128, 128]),
        "yT": nc.dram_tensor("yT", [C, T], F32, kind="ExternalOutput").ap(),
    }
    return io


def build(num_devices=N_CORES, repeats=1):
    nc = bacc.Bacc(
        "TRN2", target_bir_lowering=False, debug=False, num_devices=num_devices
    )
    # Pin Exp to the natural_log_exp table set (shared with Ln): the
    # default per-function set choice thrashes ACT_TABLE_LOADs between
    # exp_and_others and natural_log_exp on every LayerNorm.
    import concourse.hw_specs as _hws

    _tabs = _hws.get_activation_tables(nc.m.arch)
    for _name in ("exp_and_others", "exp_and_friends"):
        if _name in _tabs:
            _tabs[_name].clear()
    io = declare_io(nc)
    with tile.TileContext(nc) as tc, ExitStack() as ctx:
        emit_block(ctx, nc, tc, io, repeats=repeats)
    nc.compile()
    return nc


def _w8(w_t, scale):
    """[K, M] transposed weight -> DR-paired fp8 [128, K//128, M]."""
    f8 = mybir.dt.np(F8)
    k, m = w_t.shape
    return np.ascontiguousarray(
        (w_t * scale).reshape(k // 128, 128, m).transpose(1, 0, 2)
    ).astype(f8)


def host_inputs(x_b, attn_w, attn_b, proj_w, proj_b, fc_w, fc_b, cproj_w, cproj_b,
                ln1_w, ln1_b, ln2_w, ln2_b):
    """Per-core input dict for batch element x_b [T, C]. Folds LN1 w/b
    into wqkv/biases and LN2 w/b into wfc/biases."""
    f = np.float32
    wqkv = attn_w * ln1_w[None, :]
    bqkv = attn_b + attn_w @ ln1_b
    wfc = fc_w * ln2_w[None, :]
    bfc = fc_b + fc_w @ ln2_b
    return {
        "xT": np.ascontiguousarray(x_b.T, dtype=f),
        "wqkv8": _w8(wqkv.T.astype(f), SW),
        "wproj8": _w8(proj_w.T.astype(f), SW),
        "wfc8": _w8(wfc.T.astype(f), SW),
        "wcproj8": _w8(cproj_w.T.astype(f), SWC),
        "bqk": np.ascontiguousarray(
            (bqkv[: 2 * C] * F_QK).reshape(8, 128), dtype=f),
        "bv16": np.ascontiguousarray(
            (bqkv[2 * C :] * SV).reshape(1, C), dtype=f),
        "bproj": np.ascontiguousarray(proj_b.reshape(CT, 128), dtype=f),
        "bfc": np.ascontiguousarray(bfc.reshape(FT, 128), dtype=f),
        "bcproj": np.ascontiguousarray(cproj_b.reshape(CT, 128), dtype=f),
        "ones_d": np.ones((128, 128), dtype=f),
    }


def unpack_output(result_map):
    """Map one core's output tensors to the [T, C] batch element."""
    return result_map["yT"].T


_CACHED_NC = None
_LAST_RES = None


def kernel(x, ln1_w, ln1_b, attn_w, attn_b, proj_w, proj_b,
           ln2_w, ln2_b, fc_w, fc_b, cproj_w, cproj_b):
    global _CACHED_NC, _LAST_RES
    x = np.asarray(x)
    B = x.shape[0]
    assert B == N_CORES and x.shape[1] == T and x.shape[2] == C
    if _CACHED_NC is None:
        _CACHED_NC = build()
    nc = _CACHED_NC
    args = [np.asarray(a, dtype=np.float32)
            for a in (attn_w, attn_b, proj_w, proj_b, fc_w, fc_b,
                      cproj_w, cproj_b, ln1_w, ln1_b, ln2_w, ln2_b)]
    (attn_w, attn_b, proj_w, proj_b, fc_w, fc_b,
     cproj_w, cproj_b, ln1_w, ln1_b, ln2_w, ln2_b) = args
    in_maps = [
        host_inputs(x[b], attn_w, attn_b, proj_w, proj_b, fc_w, fc_b,
                    cproj_w, cproj_b, ln1_w, ln1_b, ln2_w, ln2_b)
        for b in range(B)
    ]
    res = bass_utils.run_bass_kernel_spmd(
        nc, in_maps, core_ids=list(range(N_CORES))
    )
    _LAST_RES = res
    out = np.empty((B, T, C), np.float32)
    for b in range(B):
        out[b] = unpack_output(res.results[b])
    return out
